# revision 18
# baseline (speedup 1.0000x reference)
"""Bayes-by-Backprop LSTM on 8 Trainium2 NeuronCores (Bass/Tile).

Strategy (data parallel, per sharding hint):
  - Shard batch B=256 across 8 cores (32 rows each); eps/weights replicated.
  - Host-side layout only (transpose/reshape/concat, no math):
      * merged weight rows [x(64); bias(1); h(128)] = 193, columns (gate, h)
        with gate order permuted to (i, f, o, ch) so sigmoid gates are
        contiguous (cols 0:384) and tanh gate is cols 384:512.
      * eps_w/eps_b merged the same way, laid out [193, S, 512] so DMA blocks
        are contiguous 2KB lines.
      * x transposed to [64, S, B] with a ones-row appended (bias via matmul).
  - Device per step t:
      Wn = sigma (.) eps[t]           (DVE; sigma = softplus(rho), built once)
      g  = [x_t;1]^T @ (mu_x + Wn_x) + h^T @ (mu_h + Wn_h)   (4 PE matmuls,
           PSUM accumulate; mu streamed from SBUF-resident const tiles)
      s = sigmoid(g[:, 0:384]); th_ch = tanh(g[:, 384:512])  (ACT)
      c' = i*th_ch + f*c ; th_c = tanh(c'); h' = o*th_c      (DVE/ACT)
      hT = PE-transpose(h')                                  (next step lhsT)
  - Final: out = h_S @ (out_mu + softplus(out_rho)*eps_out) + bias, bias via
    rank-1 matmul with a ones row.
"""

import os
import sys
import numpy as np

for _p in ("/opt/trn_rl_repo",):
    if _p not in sys.path and os.path.isdir(_p):
        sys.path.append(_p)

import concourse.bass as bass  # noqa: E402
import concourse.tile as tile  # noqa: E402
from concourse import mybir  # noqa: E402
from concourse import masks  # noqa: E402
from concourse.bass_utils import run_bass_kernel_spmd  # noqa: E402

F32 = mybir.dt.float32
AF = mybir.ActivationFunctionType
N_CORES = 8
GPERM = [0, 1, 3, 2]  # gate column order: i, f, o, ch

# The walrus bundled in this container rejects instructions carrying more
# than 2 semaphore-wait commands; Tile freely emits 3+. Split the excess
# onto same-engine NOPs inserted immediately before the instruction.
MAX_INST_WAITS = 1


def _split_excess_waits(nc, max_waits=MAX_INST_WAITS):
    blocks = []
    for f in nc.m.functions:
        for blk in f.blocks:
            blocks.append((blk, list(blk.instructions)))

    plans = {}  # id(inst) -> list of nop mybir instructions to insert before
    for blk, insts in blocks:
        for inst in insts:
            si = inst.sync_info
            if si is None:
                continue
            waits = list(si.on_wait)
            if len(waits) <= max_waits:
                continue
            ge = [w for w in waits if w.wait_mode == "sem-ge-imm"]
            other = [w for w in waits if w.wait_mode != "sem-ge-imm"]
            assert len(other) <= max_waits, (
                f"{inst.name}: {len(other)} non-ge waits, cannot split"
            )
            keep_n = max_waits - len(other)
            kept = other + (ge[len(ge) - keep_n :] if keep_n > 0 else [])
            excess = ge[: len(ge) - keep_n] if keep_n > 0 else ge
            eng = inst.engine
            nops = []
            for k in range(0, len(excess), max_waits):
                nop = nc.engines[eng].nop()
                nop.ins.sync_info = mybir.SyncInfo(
                    on_wait=list(excess[k : k + max_waits]), on_update=[]
                )
                nops.append(nop.ins)
            inst.sync_info = mybir.SyncInfo(
                on_wait=kept, on_update=list(si.on_update)
            )
            plans[id(inst)] = nops

    if not plans:
        return
    # Rebuild every block from its pre-creation snapshot with the nops
    # spliced in; the tail copies appended at creation time vanish because
    # snapshots predate them.
    for blk, orig in blocks:
        new = []
        for inst in orig:
            new.extend(plans.get(id(inst), ()))
            new.append(inst)
        blk.instructions = new


def _build_program(S, TB, shard, repeats=1):
    """Emit the bass program for one core (SPMD across 8).

    repeats>1 re-runs the whole recurrence from (h0,c0) that many times —
    identical output, used to measure device time above the dispatch floor.
    """
    nc = bass.Bass()
    d_eps = nc.declare_dram_parameter("epsm", [193, S, 512], F32, isOutput=False)
    d_xc = nc.declare_dram_parameter("xc", [65, S, shard], F32, isOutput=False)
    d_mu = nc.declare_dram_parameter("mum", [193, 512], F32, isOutput=False)
    d_rho = nc.declare_dram_parameter("rhom", [193, 512], F32, isOutput=False)
    d_h0T = nc.declare_dram_parameter("h0T", [128, shard], F32, isOutput=False)
    d_c0 = nc.declare_dram_parameter("c0", [shard, 128], F32, isOutput=False)
    d_owm = nc.declare_dram_parameter("owm", [128, 8], F32, isOutput=False)
    d_owr = nc.declare_dram_parameter("owr", [128, 8], F32, isOutput=False)
    d_eow = nc.declare_dram_parameter("eow", [128, 8], F32, isOutput=False)
    d_obm = nc.declare_dram_parameter("obm", [1, 8], F32, isOutput=False)
    d_obr = nc.declare_dram_parameter("obr", [1, 8], F32, isOutput=False)
    d_eob = nc.declare_dram_parameter("eob", [1, 8], F32, isOutput=False)
    d_out = nc.declare_dram_parameter("out", [shard, 8], F32, isOutput=True)

    from contextlib import ExitStack

    with tile.TileContext(nc) as tc, ExitStack() as ctx:
        singles = ctx.enter_context(tc.tile_pool(name="singles", bufs=1))
        pe1 = ctx.enter_context(tc.tile_pool(name="pe1", bufs=2))
        pe2 = ctx.enter_context(tc.tile_pool(name="pe2", bufs=2))
        px = ctx.enter_context(tc.tile_pool(name="px", bufs=2))
        _v2 = bool(os.environ.get("BASS_LSTM_V2"))
        pw1 = ctx.enter_context(tc.tile_pool(name="pw1", bufs=2 if _v2 else 3))
        pw2 = ctx.enter_context(tc.tile_pool(name="pw2", bufs=2 if _v2 else 3))
        phT = ctx.enter_context(tc.tile_pool(name="phT", bufs=3))
        psm = ctx.enter_context(tc.tile_pool(name="psm", bufs=4))
        psum_g = ctx.enter_context(
            tc.tile_pool(name="psum_g", bufs=3, space=bass.MemorySpace.PSUM)
        )
        psum_t = ctx.enter_context(
            tc.tile_pool(name="psum_t", bufs=2, space=bass.MemorySpace.PSUM)
        )
        psum_o = ctx.enter_context(
            tc.tile_pool(name="psum_o", bufs=1, space=bass.MemorySpace.PSUM)
        )

        # ---- constants in SBUF
        F32R = mybir.dt.float32r
        mu1 = singles.tile([128, 512], F32R)  # h-rows of mu
        mu2 = singles.tile([65, 512], F32R)  # x+bias rows of mu
        nc.gpsimd.dma_start(mu1[:], d_mu[65:193, :].bitcast(F32R))
        nc.gpsimd.dma_start(mu2[:], d_mu[0:65, :].bitcast(F32R))
        rho1 = singles.tile([128, 512], F32)
        rho2 = singles.tile([65, 512], F32)
        nc.gpsimd.dma_start(rho1[:], d_rho[65:193, :])
        nc.gpsimd.dma_start(rho2[:], d_rho[0:65, :])
        # softplus(x) = ln(1 + exp(x)) — no softplus table in this compiler;
        # ln+exp share one table set (natural_log_exp_and_others).
        def softplus(out_ap, in_ap, tmp_ap):
            nc.scalar.activation(tmp_ap, in_ap, AF.Exp)
            nc.vector.tensor_scalar_add(tmp_ap, tmp_ap, 1.0)
            nc.scalar.activation(out_ap, tmp_ap, AF.Ln)

        sig1 = singles.tile([128, 512], F32)
        sig2 = singles.tile([65, 512], F32)
        spt1 = singles.tile([128, 512], F32)
        spt2 = singles.tile([65, 512], F32)
        softplus(sig1[:], rho1[:], spt1[:])
        softplus(sig2[:], rho2[:], spt2[:])

        # ch-gate columns (384:512) pre-doubled: tanh(x) = 2*sigmoid(2x)-1, so
        # the ch pre-activation is computed as 2*g_ch and all four gates go
        # through one Sigmoid instruction.
        for ap in (sig1[:, 384:512], sig2[:, 384:512], mu1[:, 384:512], mu2[:, 384:512]):
            nc.vector.tensor_scalar_mul(ap, ap, 2.0)

        V2 = bool(os.environ.get("BASS_LSTM_V2"))
        if V2:
            sig1r = singles.tile([128, TB, 512], F32)
            sig2r = singles.tile([65, TB, 512], F32)
            for k in range(TB):
                nc.vector.tensor_copy(sig1r[:, k, :], sig1[:])
                nc.vector.tensor_copy(sig2r[:, k, :], sig2[:])

        ident = singles.tile([32, 32], F32)
        masks.make_identity(nc, ident[:])
        ones1f = singles.tile([1, shard], F32)
        nc.gpsimd.memset(ones1f[:], 1.0)
        ones1 = singles.tile([1, shard], F32R)
        nc.vector.tensor_copy(ones1[:], ones1f[:])

        # ---- output projection weights (built up-front: all Softplus together)
        owm = singles.tile([128, 8], F32)
        owr = singles.tile([128, 8], F32)
        eow = singles.tile([128, 8], F32)
        nc.gpsimd.dma_start(owm[:], d_owm[:])
        nc.gpsimd.dma_start(owr[:], d_owr[:])
        nc.gpsimd.dma_start(eow[:], d_eow[:])
        sow = singles.tile([128, 8], F32)
        sowt = singles.tile([128, 8], F32)
        softplus(sow[:], owr[:], sowt[:])
        wtmp = singles.tile([128, 8], F32)
        nc.vector.tensor_mul(wtmp[:], sow[:], eow[:])
        wout = singles.tile([128, 8], F32R)
        nc.vector.tensor_add(wout[:], wtmp[:], owm[:])

        obm = singles.tile([1, 8], F32)
        obr = singles.tile([1, 8], F32)
        eob = singles.tile([1, 8], F32)
        nc.gpsimd.dma_start(obm[:], d_obm[:])
        nc.gpsimd.dma_start(obr[:], d_obr[:])
        nc.gpsimd.dma_start(eob[:], d_eob[:])
        sob = singles.tile([1, 8], F32)
        sobt = singles.tile([1, 8], F32)
        softplus(sob[:], obr[:], sobt[:])
        btmp = singles.tile([1, 8], F32)
        nc.vector.tensor_mul(btmp[:], sob[:], eob[:])
        bout = singles.tile([1, 8], F32R)
        nc.vector.tensor_add(bout[:], btmp[:], obm[:])

        # ---- state
        cst = singles.tile([shard, 128], F32)  # cell state
        assert S % TB == 0
        e1 = e2 = xb = None
        hT = None
        for rep in range(repeats):
          nc.gpsimd.dma_start(cst[:], d_c0[:])
          hT = phT.tile([128, shard], F32R)
          nc.gpsimd.dma_start(hT[:], d_h0T[:].bitcast(F32R))

          # ---- recurrence
          for t in range(S):
            tl = t % TB
            if tl == 0:
                t0 = t
                e1 = pe1.tile([128, TB, 512], F32)
                nc.sync.dma_start(e1[:], d_eps[65:193, t0 : t0 + TB, :])
                e2 = pe2.tile([65, TB, 512], F32)
                nc.sync.dma_start(e2[:], d_eps[0:65, t0 : t0 + TB, :])
                xb = px.tile([65, TB, shard], F32R)
                nc.sync.dma_start(xb[:], d_xc[:, t0 : t0 + TB, :].bitcast(F32R))
                if V2:
                    w1b = pw1.tile([128, TB, 512], F32R)
                    nc.vector.tensor_mul(w1b[:], sig1r[:], e1[:])
                    w2b = pw2.tile([65, TB, 512], F32R)
                    nc.vector.tensor_mul(w2b[:], sig2r[:], e2[:])

            if V2:
                w1 = w1b[:, tl, :]
                w2 = w2b[:, tl, :]
            else:
                w1t = pw1.tile([128, 512], F32R)
                nc.vector.tensor_mul(w1t[:], sig1[:], e1[:, tl, :])
                w2t = pw2.tile([65, 512], F32R)
                nc.vector.tensor_mul(w2t[:], sig2[:], e2[:, tl, :])
                w1 = w1t[:]
                w2 = w2t[:]

            g = psum_g.tile([shard, 512], F32)
            xlt = xb[:, tl, :]
            nc.tensor.matmul(g[:], xlt, mu2[:], start=True, stop=False)
            nc.tensor.matmul(g[:], xlt, w2, start=False, stop=False)
            nc.tensor.matmul(g[:], hT[:], mu1[:], start=False, stop=False)
            nc.tensor.matmul(g[:], hT[:], w1, start=False, stop=True)

            s = psm.tile([shard, 512], F32)
            nc.scalar.activation(s[:], g[:], AF.Sigmoid)

            # i*tanh(g_ch)/2 = s_i * (s_ch' - 0.5) where s_ch' = sigmoid(2 g_ch)
            v2 = psm.tile([shard, 128], F32)
            nc.vector.scalar_tensor_tensor(
                v2[:], s[:, 384:512], -0.5, s[:, 0:128],
                mybir.AluOpType.add, mybir.AluOpType.mult,
            )
            fc = psm.tile([shard, 128], F32)
            nc.vector.tensor_mul(fc[:], s[:, 128:256], cst[:])
            nc.vector.scalar_tensor_tensor(
                cst[:], v2[:], 2.0, fc[:],
                mybir.AluOpType.mult, mybir.AluOpType.add,
            )
            th = psm.tile([shard, 128], F32)
            nc.scalar.activation(th[:], cst[:], AF.Tanh)
            h = psm.tile([shard, 128], F32)
            nc.vector.tensor_mul(h[:], s[:, 256:384], th[:])

            hps = psum_t.tile([128, shard], F32)
            nc.tensor.transpose(hps[:], h[:], ident[:])
            hT = phT.tile([128, shard], F32R)
            nc.scalar.copy(hT[:], hps[:])

        # ---- output projection
        ops = psum_o.tile([shard, 8], F32)
        nc.tensor.matmul(ops[:], hT[:], wout[:], start=True, stop=False)
        nc.tensor.matmul(ops[:], ones1[:], bout[:], start=False, stop=True)
        osb = singles.tile([shard, 8], F32)
        nc.vector.tensor_copy(osb[:], ops[:])
        nc.gpsimd.dma_start(d_out[:], osb[:])

    predicted_ns = None
    try:
        ent = tc._perfetto_entries
        if ent:
            # entries: (tile_name, allocated_ns, freed_ns, space, bytes, addr, tag)
            predicted_ns = int(
                max(max(e[1] or 0, e[2] or 0) for e in ent)
            )
    except Exception:
        pass
    return nc, predicted_ns


def _host_layout(inputs):
    x = np.asarray(inputs["x"], np.float32)
    h0 = np.asarray(inputs["h0"], np.float32)
    c0 = np.asarray(inputs["c0"], np.float32)
    w_mu = np.asarray(inputs["w_mu"], np.float32)
    w_rho = np.asarray(inputs["w_rho"], np.float32)
    b_mu = np.asarray(inputs["b_mu"], np.float32)
    b_rho = np.asarray(inputs["b_rho"], np.float32)
    eps_w = np.asarray(inputs["eps_w"], np.float32)
    eps_b = np.asarray(inputs["eps_b"], np.float32)

    B, S, I = x.shape
    H = h0.shape[1]
    G = 4
    GH = G * H

    def merge_rows(w_g, b_g):  # w_g [G, I+H, H], b_g [G, H] -> [I+1+H, G*H]
        rows = np.transpose(w_g, (1, 0, 2)).reshape(I + H, GH)
        brow = b_g.reshape(1, GH)
        return np.concatenate([rows[:I], brow, rows[I:]], axis=0)

    mu_m = np.ascontiguousarray(merge_rows(w_mu[GPERM], b_mu[GPERM]))
    rho_m = np.ascontiguousarray(merge_rows(w_rho[GPERM], b_rho[GPERM]))

    ew = eps_w[:, GPERM]  # [S, G, I+H, H]
    eps_rows = np.transpose(ew, (0, 2, 1, 3)).reshape(S, I + H, GH)
    eb_row = eps_b[:, GPERM].reshape(S, 1, GH)
    eps_m = np.concatenate([eps_rows[:, :I], eb_row, eps_rows[:, I:]], axis=1)
    epsT = np.ascontiguousarray(np.transpose(eps_m, (1, 0, 2)))  # [193, S, GH]

    xT = np.transpose(x, (2, 1, 0)).astype(np.float32)  # [I, S, B]
    ones_row = np.ones((1, S, B), np.float32)
    xc_all = np.concatenate([xT, ones_row], axis=0)  # [I+1, S, B]

    h0T = np.ascontiguousarray(h0.T)  # [H, B]

    ow_m = np.asarray(inputs["out_w_mu"], np.float32)
    ow_r = np.asarray(inputs["out_w_rho"], np.float32)
    e_ow = np.asarray(inputs["eps_out_w"], np.float32)
    ob_m = np.asarray(inputs["out_b_mu"], np.float32).reshape(1, -1)
    ob_r = np.asarray(inputs["out_b_rho"], np.float32).reshape(1, -1)
    e_ob = np.asarray(inputs["eps_out_b"], np.float32).reshape(1, -1)

    return dict(
        S=S, B=B, epsT=epsT, mu_m=mu_m, rho_m=rho_m, xc_all=xc_all,
        h0T=h0T, c0=c0, ow_m=ow_m, ow_r=ow_r, e_ow=e_ow,
        ob_m=ob_m, ob_r=ob_r, e_ob=e_ob,
    )


def prepare(_repeats=1, **inputs):
    """Build the bass program + per-core input maps. Returns (nc, in_maps, shard, predicted_ns)."""
    L = _host_layout(inputs)
    S, B = L["S"], L["B"]
    assert B % N_CORES == 0
    shard = B // N_CORES
    import os as _os
    if _os.environ.get("BASS_LSTM_V2"):
        TB = 8 if S % 8 == 0 else 1
    else:
        TB = 16 if S % 16 == 0 else (8 if S % 8 == 0 else 1)

    nc, predicted_ns = _build_program(S, TB, shard, repeats=_repeats)
    _split_excess_waits(nc)
    if predicted_ns and os.environ.get("BASS_LSTM_VERBOSE"):
        print(f"[kernel] tile-predicted makespan: {predicted_ns} ns")

    in_maps = []
    for c in range(N_CORES):
        sl = slice(c * shard, (c + 1) * shard)
        in_maps.append(
            {
                "epsm": L["epsT"],
                "xc": np.ascontiguousarray(L["xc_all"][:, :, sl]),
                "mum": L["mu_m"],
                "rhom": L["rho_m"],
                "h0T": np.ascontiguousarray(L["h0T"][:, sl]),
                "c0": np.ascontiguousarray(L["c0"][sl]),
                "owm": L["ow_m"],
                "owr": L["ow_r"],
                "eow": L["e_ow"],
                "obm": L["ob_m"],
                "obr": L["ob_r"],
                "eob": L["e_ob"],
            }
        )

    return nc, in_maps, shard, predicted_ns


def kernel(**inputs):
    nc, in_maps, shard, _pred = prepare(**inputs)
    res = run_bass_kernel_spmd(nc, in_maps, list(range(N_CORES)), trace=False)
    out = np.concatenate(
        [res.results[c]["out"] for c in range(N_CORES)], axis=0
    ).astype(np.float32)
    return out
